# revision 1
# baseline (speedup 1.0000x reference)
"""DiT block kernel for 8 Trainium2 NeuronCores (Bass/Tile, SPMD).

Sharding: core c = 4*b + j handles batch b (2 groups of 4 cores) and owns
token quarter j (512 tokens). Host prep per core:
  - x[b] is transposed AND rolled by -512*j tokens so each core's own
    tokens sit at columns 0:512 of its xT (attention over the full
    sequence is permutation invariant, so rolling keys/values is safe).
  - K/V are computed for the whole 2048-token batch on every core
    (redundant compute, zero communication).
  - MLP weights are replicated (streamed from HBM, bf16).
  - w_ss2 (85MB) is column-sharded 4-way inside each group; the partial
    t_emb columns are exchanged with one tiny AllGather.
All matmuls run in bf16 with fp32 PSUM accumulation; residual stream and
layernorm statistics are fp32.
"""
import sys
sys.path.insert(0, "/opt/trn_rl_repo")

import numpy as np
import ml_dtypes

import concourse.bass as bass
import concourse.tile as tile
from concourse import bacc, mybir
from concourse.bass_utils import run_bass_kernel_spmd
from concourse.masks import make_identity

P = 128
H = 768
NH = 12
HD = 64
B = 2
T = 2048
TOK = 512            # own tokens per core
KT6 = H // P         # 6 k-tiles over hidden
TT16 = T // P        # 16 token tiles over full batch
MT4 = TOK // P       # 4 token tiles over own tokens
FF = 3072
FFT = FF // P        # 24
SS = 6 * H           # 4608
SSH = SS // 4        # 1152 per-core ss2 column shard
SCALE = float(1.0 / np.sqrt(H))
EPS = 1e-5

BF = mybir.dt.bfloat16
F32 = mybir.dt.float32
AF = mybir.ActivationFunctionType
ALU = mybir.AluOpType

N_CORES = 8
ATTN_VARIANT = "full"  # full | even | nonorm
LN2_MODE = "full"  # stats | xn | full
LNBC_MEMSET = False
MLP2_SINGLE = False
STAGE = 6  # emit stages up to this number (1=ss, 2=ln1, 3=qkv, 4=attn, 5=mffn+ln2, 6=all)
SIM_SAFE = False  # replace Gelu (unimplemented in CoreSim) with Tanh for sim runs
GROUPS = [[0, 1, 2, 3], [4, 5, 6, 7]]


def _bcast(ap, p=P):
    """[N] (or [1,N]) AP -> [p, N] partition-broadcast AP (for DMA input)."""
    a = list(ap.ap)
    if len(a) == 2 and a[0][1] == 1:
        a = a[1:]
    return bass.AP(tensor=ap.tensor, offset=ap.offset, ap=[[0, p]] + a)


def _emit(ctx, tc, io):
    nc = tc.nc

    const = ctx.enter_context(tc.tile_pool(name="const", bufs=1))
    psum_big = ctx.enter_context(tc.tile_pool(name="psum_big", bufs=4, space="PSUM"))
    psum_o = ctx.enter_context(tc.tile_pool(name="psum_o", bufs=2, space="PSUM"))
    psum_t = ctx.enter_context(tc.tile_pool(name="psum_t", bufs=2, space="PSUM"))
    dram = ctx.enter_context(tc.tile_pool(name="dram", bufs=2, space="DRAM"))
    wrk = ctx.enter_context(tc.tile_pool(name="wrk", bufs=6))
    wrk768 = ctx.enter_context(tc.tile_pool(name="wrk768", bufs=8))
    small = ctx.enter_context(tc.tile_pool(name="small", bufs=8))
    wk = ctx.enter_context(tc.tile_pool(name="wk", bufs=14))
    wn = ctx.enter_context(tc.tile_pool(name="wn", bufs=6))
    eP = ctx.enter_context(tc.tile_pool(name="eP", bufs=4))

    # ---------- constants ----------
    ones_bf = const.tile([P, P], BF, name="ones_bf")
    nc.vector.memset(ones_bf[:], 1.0)
    ones_f32 = const.tile([P, P], F32, name="ones_f32")
    nc.vector.memset(ones_f32[:], 1.0)
    idn = const.tile([P, P], F32, name="idn")
    make_identity(nc, idn[:])
    eps_ap = const.tile([P, 1], F32, name="eps")
    nc.vector.memset(eps_ap[:], EPS)

    ln1g_c = const.tile([P, KT6], F32, name="ln1g")
    nc.sync.dma_start(ln1g_c[:], io["ln1g_c"][:])
    ln1b_c = const.tile([P, KT6], F32, name="ln1b")
    nc.sync.dma_start(ln1b_c[:], io["ln1b_c"][:])

    # ---------- scale_shift (adaLN) path ----------
    tT_sb = const.tile([P, KT6], BF, name="tT")
    nc.sync.dma_start(tT_sb[:], io["tT"].rearrange("(k p) o -> p (k o)", p=P))
    silu_row = const.tile([1, SS], BF, name="silu_row")
    for n in range(SS // 512):      # 9 chunks
        ps = psum_big.tile([P, 512], F32, name="pbig")[0:1, :]
        for k in range(KT6):
            w_t = wn.tile([P, 512], BF, name="wn")
            nc.sync.dma_start(w_t[:], io["ss1"][P * k:P * (k + 1), 512 * n:512 * (n + 1)])
            nc.tensor.matmul(ps, tT_sb[:, k:k + 1], w_t[:],
                             start=(k == 0), stop=(k == KT6 - 1))
        sig = wrk.tile([P, 512], F32, name="w512")[0:1, :]
        nc.scalar.activation(sig, ps, AF.Sigmoid)
        nc.vector.tensor_mul(silu_row[:, 512 * n:512 * (n + 1)], ps, sig)

    # silu row -> column layout [128, 36] via a DRAM bounce (cross-partition)
    silu_dram = dram.tile([1, SS], BF)
    nc.sync.dma_start(silu_dram[:], silu_row[:])
    silu_cols = const.tile([P, SS // P], BF, name="silu_cols")
    nc.sync.dma_start(silu_cols[:], silu_dram.rearrange("o (k p) -> (o p) k", p=P))

    # t_emb shard [1, 1152]: contraction over 4608
    temb_sh = const.tile([1, SSH], F32, name="temb_sh")
    for (n0, nsz) in [(0, 512), (512, 512), (1024, 128)]:
        ps = psum_big.tile([P, 512], F32, name="pbig")[0:1, 0:nsz]
        for k in range(SS // P):    # 36
            w_t = wn.tile([P, 512], BF, name="wn")[:, 0:nsz]
            nc.sync.dma_start(w_t, io["ss2s"][P * k:P * (k + 1), n0:n0 + nsz])
            nc.tensor.matmul(ps, silu_cols[:, k:k + 1], w_t,
                             start=(k == 0), stop=(k == SS // P - 1))
        nc.vector.tensor_copy(temb_sh[:, n0:n0 + nsz], ps)

    cc_in = dram.tile([1, SSH], F32)
    cc_out = dram.tile([4, SSH], F32)
    nc.sync.dma_start(cc_in[:], temb_sh[:])
    nc.gpsimd.collective_compute(
        "AllGather", ALU.bypass, replica_groups=GROUPS,
        ins=[cc_in.opt()], outs=[cc_out.opt()],
    )
    cc_flat = cc_out.rearrange("r i -> (r i)")
    # t_emb columns for LN1: [128, 36], feature f = 128*j + p
    temb_cols = const.tile([P, SS // P], F32, name="temb_cols")
    nc.sync.dma_start(temb_cols[:], cc_out.rearrange("r (k p) -> p (r k)", p=P))
    g1_cols = temb_cols[:, 0:KT6]
    be1_cols = temb_cols[:, KT6:2 * KT6]

    # modulation constants (LN1 in column layout)
    G1c = const.tile([P, KT6], F32, name="G1c")
    nc.vector.tensor_mul(G1c[:], g1_cols, ln1g_c[:])
    B1c = const.tile([P, KT6], F32, name="B1c")
    nc.vector.tensor_mul(B1c[:], g1_cols, ln1b_c[:])
    nc.vector.tensor_add(B1c[:], B1c[:], be1_cols)

    # broadcast tiles for the normal-layout stages (rows of t_emb)
    A1bc = const.tile([P, H], F32, name="A1bc")
    nc.sync.dma_start(A1bc[:], _bcast(cc_flat[2 * H:3 * H]))
    A2bc = const.tile([P, H], F32, name="A2bc")
    nc.sync.dma_start(A2bc[:], _bcast(cc_flat[5 * H:6 * H]))
    g2raw = wrk768.tile([P, H], F32, name="w768")
    nc.sync.dma_start(g2raw[:], _bcast(cc_flat[3 * H:4 * H]))
    be2raw = wrk768.tile([P, H], F32, name="w768")
    nc.sync.dma_start(be2raw[:], _bcast(cc_flat[4 * H:5 * H]))
    ln2g_bc = wrk768.tile([P, H], F32, name="w768")
    ln2b_bc = wrk768.tile([P, H], F32, name="w768")
    nc.sync.dma_start(ln2g_bc[:], io["ln2g_bc"][:])
    nc.sync.dma_start(ln2b_bc[:], io["ln2b_bc"][:])
    # G2bc = g2 * ln2_g ; B2bc = g2 * ln2_b + be2
    G2bc = const.tile([P, H], F32, name="G2bc")
    nc.vector.tensor_mul(G2bc[:], g2raw[:], ln2g_bc[:])
    B2bc = const.tile([P, H], F32, name="B2bc")
    nc.vector.tensor_mul(B2bc[:], g2raw[:], ln2b_bc[:])
    nc.vector.tensor_add(B2bc[:], B2bc[:], be2raw[:])

    # ---------- stage 1: LN1 (transposed layout, full batch) ----------
    if STAGE < 2:
        dbg = xp_dbg(tc, io, nc, temb_cols)
        return
    hT_cm = tc.tile_pool(name="hTp", bufs=1, side="right")
    hTp = hT_cm.__enter__()
    hT = hTp.tile([P, KT6, T], BF, name="hT")

    early_cm = tc.tile_pool(name="early", bufs=1)
    early = early_cm.__enter__()
    sqp_cm = tc.tile_pool(name="sqp", bufs=3)
    sqp = sqp_cm.__enter__()

    xT_sb = early.tile([P, KT6, T], BF, name="xT")
    for k in range(KT6):
        nc.sync.dma_start(xT_sb[:, k, :], io["xT"][P * k:P * (k + 1), :])
    c1t = early.tile([P, T], F32, name="c1t")
    c0t = early.tile([P, T], F32, name="c0t")
    for n in range(T // 512):
        ns = slice(512 * n, 512 * (n + 1))
        ps_mu = psum_big.tile([P, 512], F32, name="pbig")
        ps_sq = psum_big.tile([P, 512], F32, name="pbig")
        for k in range(KT6):
            xsq = sqp.tile([P, 512], F32, name="xsq")
            nc.scalar.activation(xsq[:], xT_sb[:, k, ns], AF.Square)
            nc.tensor.matmul(ps_mu[:], ones_bf[:], xT_sb[:, k, ns],
                             start=(k == 0), stop=(k == KT6 - 1))
            nc.tensor.matmul(ps_sq[:], ones_f32[:], xsq[:],
                             start=(k == 0), stop=(k == KT6 - 1))
        mu = wrk.tile([P, 512], F32, name="w512")
        nc.vector.tensor_scalar(mu[:], ps_mu[:], 1.0 / H, None, ALU.mult)
        musq = wrk.tile([P, 512], F32, name="w512")
        nc.vector.tensor_mul(musq[:], mu[:], mu[:])
        varme = wrk.tile([P, 512], F32, name="w512")
        nc.vector.scalar_tensor_tensor(varme[:], ps_sq[:], 1.0 / H, musq[:],
                                       ALU.mult, ALU.subtract)
        std = wrk.tile([P, 512], F32, name="w512")
        nc.scalar.activation(std[:], varme[:], AF.Sqrt, bias=eps_ap[:])
        nc.vector.reciprocal(c1t[:, ns], std[:])
        nc.vector.tensor_mul(c0t[:, ns], mu[:], c1t[:, ns])
    # apply: h = (x*c1 - c0) * G1[k] + B1[k]
    for k in range(KT6):
        for n in range(T // 512):
            ns = slice(512 * n, 512 * (n + 1))
            xn = wrk.tile([P, 512], F32, name="w512")
            nc.vector.tensor_mul(xn[:], xT_sb[:, k, ns], c1t[:, ns])
            nc.vector.tensor_sub(xn[:], xn[:], c0t[:, ns])
            nc.vector.tensor_scalar(hT[:, k, ns], xn[:],
                                    G1c[:, k:k + 1], B1c[:, k:k + 1],
                                    ALU.mult, ALU.add)
    sqp_cm.__exit__(None, None, None)
    early_cm.__exit__(None, None, None)

    if STAGE < 3:
        nc.sync.dma_start(io["out"].rearrange("(a p) f -> p a f", p=P)[:, 0:KT6//2, 0:T//2].rearrange("p a f -> p (a f)"), hT[:, 0, :].rearrange("p t -> p () t").rearrange("p o t -> p (o t)"))
        hT_cm.__exit__(None, None, None)
        return
    # ---------- stage 2: qkv ----------
    att_cm = tc.tile_pool(name="attp", bufs=1)
    attp = att_cm.__enter__()
    KTs = attp.tile([P, KT6, T], BF, name="KTs")
    QTs = attp.tile([P, KT6, TOK], BF, name="QTs")
    V_aug = attp.tile([P, TT16, NH, HD + 1], BF, name="Vaug")
    nc.vector.memset(V_aug[:, :, :, HD:HD + 1], 1.0)

    qkv_cm = tc.tile_pool(name="qkvw", bufs=1, side="right")
    qkvw = qkv_cm.__enter__()
    Wqkv = qkvw.tile([P, KT6, 3 * H], BF, name="Wqkv")
    for k in range(KT6):
        nc.sync.dma_start(Wqkv[:, k, :], io["wqkv"][P * k:P * (k + 1), :])

    # K^T (full batch)
    for m in range(KT6):
        for n in range(T // 512):
            ns = slice(512 * n, 512 * (n + 1))
            ps = psum_big.tile([P, 512], F32, name="pbig")
            for k in range(KT6):
                nc.tensor.matmul(ps[:], Wqkv[:, k, H + P * m:H + P * (m + 1)],
                                 hT[:, k, ns], start=(k == 0), stop=(k == KT6 - 1))
            nc.vector.tensor_copy(KTs[:, m, ns], ps[:])
    # Q^T (own tokens)
    for m in range(KT6):
        ps = psum_big.tile([P, 512], F32, name="pbig")
        for k in range(KT6):
            nc.tensor.matmul(ps[:], Wqkv[:, k, P * m:P * (m + 1)],
                             hT[:, k, 0:TOK], start=(k == 0), stop=(k == KT6 - 1))
        nc.vector.tensor_copy(QTs[:, m, :], ps[:])
    # V (normal layout, full batch) + ones column
    for mt in range(TT16):
        msl = slice(P * mt, P * (mt + 1))
        for (n0, nsz) in [(0, 512), (512, 256)]:
            ps = psum_big.tile([P, 512], F32, name="pbig")[:, 0:nsz]
            for k in range(KT6):
                nc.tensor.matmul(ps, hT[:, k, msl],
                                 Wqkv[:, k, 2 * H + n0:2 * H + n0 + nsz],
                                 start=(k == 0), stop=(k == KT6 - 1))
            h0 = n0 // HD
            nc.vector.tensor_copy(
                V_aug[:, mt, h0:h0 + nsz // HD, 0:HD],
                ps.rearrange("p (h d) -> p h d", d=HD))
    qkv_cm.__exit__(None, None, None)
    hT_cm.__exit__(None, None, None)

    if STAGE < 4:
        att_cm.__exit__(None, None, None)
        return
    # ---------- stage 3: attention ----------
    oT_cm = tc.tile_pool(name="oTp", bufs=1, side="right")
    oTp = oT_cm.__enter__()
    oT = oTp.tile([P, KT6, TOK], BF, name="oT")
    heads = range(NH) if ATTN_VARIANT != "even" else range(0, NH, 2)
    for h in heads:
        h_t = h // 2
        off = HD * (h % 2)
        if ATTN_VARIANT == "even":
            off = 0
        ps_o = psum_o.tile([HD + 1, 512], F32, name="po")
        for kt in range(TT16):
            ps_s = psum_big.tile([P, 512], F32, name="pbig")
            nc.tensor.matmul(ps_s[:],
                             KTs[off:off + HD, h_t, P * kt:P * (kt + 1)],
                             QTs[off:off + HD, h_t, :],
                             start=True, stop=True)
            e_t = eP.tile([P, 512], BF, name="e")
            nc.scalar.activation(e_t[:], ps_s[:], AF.Exp, scale=SCALE)
            nc.tensor.matmul(ps_o[:], V_aug[:, kt, h, :], e_t[:],
                             start=(kt == 0), stop=(kt == TT16 - 1))
        if ATTN_VARIANT == "nonorm":
            o_st = wrk.tile([P, 512], BF, name="ost")[0:HD, :]
            nc.vector.tensor_copy(o_st, ps_o[0:HD, :])
            nc.sync.dma_start(oT[off:off + HD, h_t, :], o_st)
            continue
        # sums live on psum partition 64; DVE lanes are partition-locked, so
        # move the row to partition 0 with a DMA before reciprocal/broadcast.
        s_st = wrk.tile([P, 512], F32, name="w512")[HD:HD + 1, :]
        nc.vector.tensor_copy(s_st, ps_o[HD:HD + 1, :])
        rec = small.tile([1, 512], F32, name="rec")
        nc.sync.dma_start(rec[:], s_st)
        nc.vector.reciprocal(rec[:], rec[:])
        recbc = wrk.tile([P, 512], F32, name="w512")[0:HD, :]
        nc.gpsimd.partition_broadcast(recbc, rec[:])
        if off == 0:
            nc.vector.tensor_mul(oT[0:HD, h_t, :], ps_o[0:HD, :], recbc)
        else:
            o_st = wrk.tile([P, 512], BF, name="ost")[0:HD, :]
            nc.vector.tensor_mul(o_st, ps_o[0:HD, :], recbc)
            nc.sync.dma_start(oT[off:off + HD, h_t, :], o_st)
    att_cm.__exit__(None, None, None)

    if STAGE < 5:
        oT_cm.__exit__(None, None, None)
        return
    # ---------- stages 4+6: the two MLPs ----------
    xp_cm = tc.tile_pool(name="xp", bufs=1)
    xp = xp_cm.__enter__()
    gT_cm = tc.tile_pool(name="gTp", bufs=1)
    gTp = gT_cm.__enter__()

    x1 = xp.tile([P, MT4, H], F32, name="x1")
    xown = xp.tile([P, MT4, H], F32, name="xbuf")
    nc.sync.dma_start(xown[:], io["xown"].rearrange("(mt p) f -> p mt f", p=P))

    def mlp(inT, w1_dram, w2_dram, abc, res_in, out_tile):
        gT = gTp.tile([P, FFT, TOK], BF, name="gT")
        for m in range(FFT):
            ps = psum_big.tile([P, 512], F32, name="pbig")
            for k in range(KT6):
                w_t = wk.tile([P, P], BF, name="wk1")
                nc.sync.dma_start(w_t[:], w1_dram[P * k:P * (k + 1), P * m:P * (m + 1)])
                nc.tensor.matmul(ps[:], w_t[:], inT[:, k, :],
                                 start=(k == 0), stop=(k == KT6 - 1))
            nc.scalar.activation(gT[:, m, :], ps[:], AF.Tanh if SIM_SAFE else AF.Gelu)
        if MLP2_SINGLE:
            for mt in range(MT4):
                for (n0, nsz) in [(0, 512), (512, 256)]:
                    ps1 = psum_big.tile([P, 512], F32, name="pbig")[:, 0:nsz]
                    for k in range(FFT):
                        w_t = wn.tile([P, 512], BF, name="wn")[:, 0:nsz]
                        nc.sync.dma_start(w_t, w2_dram[P * k:P * (k + 1), n0:n0 + nsz])
                        nc.tensor.matmul(ps1, gT[:, k, P * mt:P * (mt + 1)], w_t,
                                         start=(k == 0), stop=(k == FFT - 1))
                    tmp = wrk.tile([P, 512], F32, name="w512")[:, 0:nsz]
                    nc.vector.tensor_mul(tmp, ps1, abc[:, n0:n0 + nsz])
                    nc.vector.tensor_add(out_tile[:, mt, n0:n0 + nsz],
                                         res_in[:, mt, n0:n0 + nsz], tmp)
            return
        for (n0, nsz) in [(0, 512), (512, 256)]:
            ps_l = [psum_big.tile([P, 512], F32, name="pbig")[:, 0:nsz]
                    for _ in range(MT4)]
            for k in range(FFT):
                w_t = wn.tile([P, 512], BF, name="wn")[:, 0:nsz]
                nc.sync.dma_start(w_t, w2_dram[P * k:P * (k + 1), n0:n0 + nsz])
                for mt in range(MT4):
                    nc.tensor.matmul(ps_l[mt], gT[:, k, P * mt:P * (mt + 1)], w_t,
                                     start=(k == 0), stop=(k == FFT - 1))
            for mt in range(MT4):
                tmp = wrk.tile([P, 512], F32, name="w512")[:, 0:nsz]
                nc.vector.tensor_mul(tmp, ps_l[mt], abc[:, n0:n0 + nsz])
                nc.vector.tensor_add(out_tile[:, mt, n0:n0 + nsz],
                                     res_in[:, mt, n0:n0 + nsz], tmp)

    if STAGE == 41:
        gT41 = gTp.tile([P, FFT, TOK], BF, name="gT")
        for m in range(FFT):
            ps41 = psum_big.tile([P, 512], F32, name="pbig")
            for k in range(KT6):
                w_t41 = wk.tile([P, P], BF, name="wk1")
                nc.sync.dma_start(w_t41[:], io["wm1"][P * k:P * (k + 1), P * m:P * (m + 1)])
                nc.tensor.matmul(ps41[:], w_t41[:], oT[:, k, :],
                                 start=(k == 0), stop=(k == KT6 - 1))
            nc.scalar.activation(gT41[:, m, :], ps41[:], AF.Tanh if SIM_SAFE else AF.Gelu)
        oT_cm.__exit__(None, None, None)
        gT_cm.__exit__(None, None, None)
        xp_cm.__exit__(None, None, None)
        return
    if LN2_MODE.startswith("isolate"):
        nc.vector.tensor_copy(x1[:], xown[:])
    else:
        mlp(oT, io["wm1"], io["wm2"], A1bc, xown, x1)
    oT_cm.__exit__(None, None, None)
    if STAGE == 42:
        nc.sync.dma_start(io["out"].rearrange("(mt p) f -> p mt f", p=P), x1[:])
        gT_cm.__exit__(None, None, None)
        xp_cm.__exit__(None, None, None)
        return

    # ---------- stage 5: LN2 + modulation + transpose ----------
    h2 = xp.tile([P, MT4, H], F32, name="h2")
    SUB = 256
    NSUB = H // SUB
    for mt in range(MT4):
        xin = x1[:, mt, :].rearrange("p (s f) -> p s f", f=SUB)
        stats = wrk768.tile([P, NSUB, nc.vector.BN_STATS_DIM], F32, name="bnst")
        for s in range(NSUB):
            nc.vector.bn_stats(out=stats[:, s, :], in_=xin[:, s, :])
        mv = small.tile([P, nc.vector.BN_AGGR_DIM], F32, name="mv")
        nc.vector.bn_aggr(out=mv[:], in_=stats[:])
        rstd = small.tile([P, 1], F32, name="s7")
        nc.scalar.activation(rstd[:], mv[:, 1:2], AF.Sqrt, bias=eps_ap[:])
        nc.vector.reciprocal(rstd[:], rstd[:])
        xn2 = wrk768.tile([P, H], F32, name="w768")
        nc.vector.tensor_scalar(xn2[:], x1[:, mt, :], mv[:, 0:1], rstd[:],
                                ALU.subtract, ALU.mult)
        if LN2_MODE.endswith("xn"):
            nc.vector.tensor_copy(h2[:, mt, :], xn2[:])
            continue
        t2 = wrk768.tile([P, H], F32, name="w768")
        nc.vector.tensor_mul(t2[:], xn2[:], G2bc[:])
        nc.vector.tensor_add(h2[:, mt, :], t2[:], B2bc[:])

    if STAGE == 45:
        nc.sync.dma_start(io["out"].rearrange("(mt p) f -> p mt f", p=P), h2[:])
        gT_cm.__exit__(None, None, None)
        xp_cm.__exit__(None, None, None)
        return
    h2T = xp.tile([P, KT6, TOK], BF, name="h2T")
    for mt in range(MT4):
        for k in range(KT6):
            pst = psum_t.tile([P, P], F32, name="pt")
            nc.tensor.transpose(pst[:], h2[:, mt, P * k:P * (k + 1)], idn[:])
            nc.vector.tensor_copy(h2T[:, k, P * mt:P * (mt + 1)], pst[:])

    if STAGE < 6:
        nc.sync.dma_start(io["out"].rearrange("(mt p) f -> p mt f", p=P), x1[:])
        gT_cm.__exit__(None, None, None)
        xp_cm.__exit__(None, None, None)
        return
    # ---------- stage 6: FFN ----------
    out_sb = xp.tile([P, MT4, H], F32, name="outb")
    mlp(h2T, io["wf1"], io["wf2"], A2bc, x1, out_sb)
    nc.sync.dma_start(io["out"].rearrange("(mt p) f -> p mt f", p=P), out_sb[:])

    gT_cm.__exit__(None, None, None)
    xp_cm.__exit__(None, None, None)


def xp_dbg(tc, io, nc, temb_cols):
    # dump temb_cols into the top-left of out for inspection
    nc.sync.dma_start(io["out"][0:P, 0:SS // P].rearrange("(o p) f -> p o f", p=P).rearrange("p o f -> p (o f)"), temb_cols[:])


_CACHE = {}


def _build():
    key = (STAGE, SIM_SAFE, ATTN_VARIANT, MLP2_SINGLE, LN2_MODE, LNBC_MEMSET)
    if key in _CACHE:
        return _CACHE[key]
    nc = bacc.Bacc("TRN2", target_bir_lowering=False, debug=False, num_devices=N_CORES)
    io = {}
    def inp(name, shape, dt):
        io[name] = nc.dram_tensor(name, shape, dt, kind="ExternalInput").ap()
    inp("xT", [H, T], BF)
    inp("xown", [TOK, H], F32)
    inp("tT", [H, 1], BF)
    inp("wqkv", [H, 3 * H], BF)
    inp("wm1", [H, FF], BF)
    inp("wm2", [FF, H], BF)
    inp("wf1", [H, FF], BF)
    inp("wf2", [FF, H], BF)
    inp("ss1", [H, SS], BF)
    inp("ss2s", [SS, SSH], BF)
    inp("ln1g_c", [P, KT6], F32)
    inp("ln1b_c", [P, KT6], F32)
    inp("ln2g_bc", [P, H], F32)
    inp("ln2b_bc", [P, H], F32)
    io["out"] = nc.dram_tensor("out", [TOK, H], F32, kind="ExternalOutput").ap()
    from contextlib import ExitStack
    with tile.TileContext(nc) as tc, ExitStack() as ctx:
        _emit(ctx, tc, io)
    nc.compile()
    _CACHE[key] = nc
    return nc


def _bf16(a):
    return np.ascontiguousarray(a.astype(ml_dtypes.bfloat16))


def make_in_maps(inputs):
    x = np.asarray(inputs["x"], np.float32)
    t = np.asarray(inputs["t"], np.float32)
    for zname in ("b_qkv", "b_mffn1", "b_mffn2", "b_ss1", "b_ss2", "b_ffn1", "b_ffn2"):
        if np.any(np.asarray(inputs[zname])):
            raise NotImplementedError(f"{zname} must be zero (kernel folds biases away)")

    wqkv = _bf16(inputs["w_qkv"])
    wm1 = _bf16(inputs["w_mffn1"])
    wm2 = _bf16(inputs["w_mffn2"])
    wf1 = _bf16(inputs["w_ffn1"])
    wf2 = _bf16(inputs["w_ffn2"])
    ss1 = _bf16(inputs["w_ss1"])
    ss2 = np.asarray(inputs["w_ss2"], np.float32)
    ln1g_c = np.ascontiguousarray(np.asarray(inputs["ln1_g"], np.float32).reshape(KT6, P).T)
    ln1b_c = np.ascontiguousarray(np.asarray(inputs["ln1_b"], np.float32).reshape(KT6, P).T)
    ln2g_bc = np.ascontiguousarray(np.broadcast_to(np.asarray(inputs["ln2_g"], np.float32).reshape(1, H), (P, H)))
    ln2b_bc = np.ascontiguousarray(np.broadcast_to(np.asarray(inputs["ln2_b"], np.float32).reshape(1, H), (P, H)))

    in_maps = []
    for c in range(N_CORES):
        b, j = divmod(c, 4)
        rolled = np.roll(x[b], -TOK * j, axis=0)
        in_maps.append({
            "xT": _bf16(rolled.T),
            "xown": np.ascontiguousarray(rolled[:TOK]),
            "tT": _bf16(t[b].reshape(H, 1)),
            "wqkv": wqkv, "wm1": wm1, "wm2": wm2, "wf1": wf1, "wf2": wf2,
            "ss1": ss1,
            "ss2s": _bf16(ss2[:, SSH * j:SSH * (j + 1)]),
            "ln1g_c": ln1g_c, "ln1b_c": ln1b_c,
            "ln2g_bc": ln2g_bc, "ln2b_bc": ln2b_bc,
        })
    return in_maps


def kernel(**inputs):
    in_maps = make_in_maps(inputs)
    nc = _build()
    res = run_bass_kernel_spmd(nc, in_maps, core_ids=list(range(N_CORES)))
    out = np.empty((B, T, H), np.float32)
    for c in range(N_CORES):
        b, j = divmod(c, 4)
        out[b, TOK * j:TOK * (j + 1)] = res.results[c]["out"]
    return out

